# revision 1
# baseline (speedup 1.0000x reference)
"""Bass/Trainium2 kernel for nn_Attention_369367188096 (sparse_attention).

Reference computation (B=2, N=4096, IN_DIM=1024, DIM=1024, HEADS=8, d=128):
    qkv = x @ W_qkv ; split into q,k,v per head
    dots = (q @ k^T) * DIM**-0.5 ; masked on top-left [2048,2048] block
    attn = softmax(dots) ; out = attn @ v ; out @ W_out + b_out

Sharding across 8 NeuronCores: core i handles batch b=i//4 and heads
(2*(i%4), 2*(i%4)+1).  Each core computes a partial output
x[b]-rows x DIM using its two heads' slice of W_out (row-sharded);
the host sums 4 partials per batch and adds b_out.

All matmul operands are bf16 (PE runs bf16 at 1 cycle/row vs 4x for fp32);
accumulation is fp32 in PSUM.  Softmax uses no max-subtraction: scores are
|s| <~ 1.5 after the 1/32 scale, so exp is numerically safe, and masking is
an exact 0/1 multiply after exp (identical to exp(-inf)=0).

Device dataflow (all layouts chosen so matmuls only ever stream, never
transpose): Q^T,K^T = W.T @ x^T with W chunks as PE weights; V natural via
x^T chunks as weights; S^T = K Q^T per (j-chunk, i-group of 512); exp on
ScalarE (scale folded in), 0/1 mask multiply on VectorE; out^T accumulates
V.T @ exp(S^T); the softmax denominator rides a ones-weights matmul whose
output is already broadcast across partitions (chunk pairs pre-summed on
VectorE off the masked region to halve those PE streams); 1/den via VectorE
reciprocal; out^T slices are exactly the lhsT the output projection needs.
"""

import os
import sys

for _p in ("/opt/trn_rl_repo", "/root/.axon_site/_ro/trn_rl_repo"):
    if os.path.isdir(_p) and _p not in sys.path:
        sys.path.insert(0, _p)

from contextlib import ExitStack

import ml_dtypes
import numpy as np

import concourse.bass as bass
import concourse.bacc as bacc
import concourse.mybir as mybir
import concourse.tile as tile
from concourse.bass_utils import run_bass_kernel_spmd

BF16 = mybir.dt.bfloat16
F32 = mybir.dt.float32
P = 128          # partitions
IN_DIM = 1024    # model in dim
OUT_DIM = 1024   # model out dim
DH = 128         # head dim
NH = 2           # heads per core
FD = 512         # matmul moving free dim
N_FULL = 4096    # sequence length
MM_FULL = 2048   # masked block size
SCALE = 1024 ** -0.5
N_CORES = 8


def build_nc(n=N_FULL, mm=MM_FULL):
    """Build the per-core Bass program (SPMD: same program, per-core data)."""
    CI = IN_DIM // P          # 8 input-dim chunks
    JC = n // P               # key chunks (32)
    IG = n // FD              # query groups of 512 (8)
    MJ = mm // P              # masked key chunks (16)
    MG = mm // FD             # masked query groups (4)
    assert MJ % 2 == 0 and JC % 2 == 0
    AF = mybir.ActivationFunctionType

    nc = bacc.Bacc("TRN2", target_bir_lowering=False, debug=False)
    # W tensors arrive host-prelayouted with 128 partitions contiguous so the
    # DMAs are dense and fast (they gate the first matmul).
    wq_d = nc.dram_tensor("wq", [P, CI * NH * DH], BF16, kind="ExternalInput")
    wk_d = nc.dram_tensor("wk", [P, CI * NH * DH], BF16, kind="ExternalInput")
    wv_d = nc.dram_tensor("wv", [P, CI * NH * DH], BF16, kind="ExternalInput")
    wo_d = nc.dram_tensor("wo", [P, NH * OUT_DIM], BF16, kind="ExternalInput")
    xt_d = nc.dram_tensor("xt", [IN_DIM, n], BF16, kind="ExternalInput")
    mk_d = nc.dram_tensor("maskt", [mm, mm], BF16, kind="ExternalInput")
    out_d = nc.dram_tensor("part", [n, OUT_DIM], F32, kind="ExternalOutput")

    xt_v = xt_d.rearrange("(c p) n -> c p n", p=P)
    mk_v = mk_d.rearrange("(j p) i -> p j i", p=P)
    out_v = out_d.rearrange("(t p) o -> t p o", p=P)

    with tile.TileContext(nc) as tc, ExitStack() as ctx:
        const = ctx.enter_context(tc.tile_pool(name="const", bufs=1))

        # Resident inputs (W first: they gate the first matmuls)
        wq = const.tile([P, CI, NH * DH], BF16, tag="wq")
        wk = const.tile([P, CI, NH * DH], BF16, tag="wk")
        wv = const.tile([P, CI, NH * DH], BF16, tag="wv")
        wo = const.tile([P, NH, OUT_DIM], BF16, tag="wo")
        for t, d_ in ((wq, wq_d), (wk, wk_d), (wv, wv_d), (wo, wo_d)):
            nc.sync.dma_start(t[:], d_.rearrange("p (a b) -> p a b", a=t.shape[1]))
        xt = [const.tile([P, n], BF16, tag=f"xt{c}", name=f"xt{c}") for c in range(CI)]
        for c in range(CI):
            nc.sync.dma_start(xt[c][:], xt_v[c])
        ones = const.tile([P, P], BF16, tag="ones")
        nc.vector.memset(ones[:], 1.0)

        # Resident intermediates
        qt = [const.tile([P, n], BF16, tag=f"qt{h}", name=f"qt{h}") for h in range(NH)]
        kt = [const.tile([P, n], BF16, tag=f"kt{h}", name=f"kt{h}") for h in range(NH)]
        vb = const.tile([P, JC, NH * DH], BF16, tag="vb")      # [j, jc, (h d)]
        ot = [const.tile([P, n], BF16, tag=f"ot{h}", name=f"ot{h}") for h in range(NH)]

        # ---- Phase 1: projections ----
        # Q^T, K^T per head: accumulate W[c,h].T @ x^T[c] over c.
        with tc.tile_pool(name="pq", bufs=4, space="PSUM") as pq:
            for h in range(NH):
                for w_sb, dst in ((wq, qt[h]), (wk, kt[h])):
                    for g0 in range(0, IG, 4):
                        gg = range(g0, min(g0 + 4, IG))
                        ps = [pq.tile([P, FD], F32, tag="pq", name="psqk") for _ in gg]
                        for c in range(CI):
                            for gi, g in enumerate(gg):
                                nc.tensor.matmul(
                                    ps[gi][:],
                                    w_sb[:, c, h * DH:(h + 1) * DH],
                                    xt[c][:, g * FD:(g + 1) * FD],
                                    start=(c == 0), stop=(c == CI - 1),
                                )
                        for gi, g in enumerate(gg):
                            nc.any.tensor_copy(dst[:, g * FD:(g + 1) * FD], ps[gi][:])
            # V (both heads) in natural [seq, d] layout: x^T[c] as weights.
            for t in range(JC):
                ps = pq.tile([P, NH * DH], F32, tag="pv")
                for c in range(CI):
                    nc.tensor.matmul(
                        ps[:], xt[c][:, t * P:(t + 1) * P], wv[:, c, :],
                        start=(c == 0), stop=(c == CI - 1),
                    )
                nc.any.tensor_copy(vb[:, t, :], ps[:])

        # ---- Phase 2: attention per head ----
        # j-chunks processed in pairs: one [P, 2*FD] exp and one mask multiply
        # per pair halves the ScalarE/VectorE per-op overhead.
        with (
            tc.tile_pool(name="pst", bufs=3, space="PSUM") as pst,
            tc.tile_pool(name="po", bufs=1, space="PSUM") as po,
            tc.tile_pool(name="pd", bufs=1, space="PSUM") as pd,
            tc.tile_pool(name="att", bufs=8) as att,
            tc.tile_pool(name="mkp", bufs=8) as mkp,
        ):
            # PSUM is the scarce resource (8 banks): st tiles get 3 slots
            # (2 banks each) so the PE can run two pairs ahead of exp; the
            # single oacc/dacc banks are released by two immediate fp32
            # copies to SBUF at i-group end.  The slow reciprocal+normalize
            # then run from the SBUF copies, emitted a few pairs into the
            # NEXT i-group: VectorE executes in order, and a 3.4us
            # RECIPROCAL at the head of its queue would block the next
            # group's mask multiplies (which gate PV matmuls -> PE stalls).
            pending = None

            def evict_den(p_dacc):
                dsb = att.tile([P, FD], F32, tag="dsb", name="dsb", bufs=2)
                nc.vector.tensor_copy(dsb[:], p_dacc[:])
                return dsb

            def finalize(pend):
                p_osb, p_dsb, p_h, p_gs = pend
                rec = att.tile([P, FD], F32, tag="rec", name="rec", bufs=2)
                nc.vector.reciprocal(rec[:], p_dsb[:])
                nc.vector.tensor_mul(
                    out=ot[p_h][:, p_gs:p_gs + FD], in0=p_osb[:], in1=rec[:],
                )

            NP2 = JC // 2
            fin_at = 10 if NP2 > 12 else NP2 - 1

            for h in range(NH):
                for g in range(IG):
                    gs = g * FD
                    oacc = po.tile([P, FD], F32, tag="po")   # [d, i] accum
                    dacc = pd.tile([P, FD], F32, tag="pd")   # bcast denom accum
                    prev_dsum = None  # for quad-summing unmasked pairs
                    first_den = True
                    for jp in range(NP2):
                        j0 = 2 * jp
                        st2 = pst.tile([P, 2, FD], F32, tag="st")
                        for u in range(2):
                            nc.tensor.matmul(
                                st2[:, u, :],
                                kt[h][:, (j0 + u) * P:(j0 + u + 1) * P],
                                qt[h][:, gs:gs + FD],
                                start=True, stop=True,
                            )
                        masked = j0 + 1 < MJ and g < MG
                        # The two pairs after the reciprocal emission point
                        # keep a zero-DVE-dependency path (direct den matmuls)
                        # so the in-order VectorE queue's 3.4us RECIPROCAL
                        # can't starve the PE through a dsum.
                        shadow = NP2 > 12 and jp in (fin_at, fin_at + 1)
                        pt2 = att.tile([P, 2, FD], BF16, tag="pt")
                        mt2 = None
                        if masked:
                            mt2 = mkp.tile([P, 2, FD], BF16, tag="mt")
                            nc.sync.dma_start(
                                mt2[:], mk_v[:, j0:j0 + 2, gs:gs + FD])
                        # Unmasked pairs: one wide exp (ScalarE per-op overhead
                        # ~172 cycles would otherwise rate-limit ACT).  Masked
                        # pairs: per-chunk exp+multiply to shorten the
                        # exp->mask->PV dependency chain the PE waits on.
                        if masked:
                            for u in range(2):
                                nc.scalar.activation(
                                    pt2[:, u, :], st2[:, u, :], AF.Exp,
                                    scale=SCALE)
                                nc.vector.tensor_mul(
                                    out=pt2[:, u, :], in0=pt2[:, u, :],
                                    in1=mt2[:, u, :])
                        else:
                            nc.scalar.activation(
                                pt2[:], st2[:], AF.Exp, scale=SCALE)
                        for u in range(2):
                            nc.tensor.matmul(
                                oacc[:], vb[:, j0 + u, h * DH:(h + 1) * DH],
                                pt2[:, u, :],
                                start=(j0 + u == 0), stop=(j0 + u == JC - 1),
                            )
                        # Denominator: a ones-weights matmul leaves the row sum
                        # already broadcast across partitions.  The [1,FD]-out
                        # stream costs a full FD cycles, so off the masked
                        # region chunk pairs are pre-summed on VectorE (idle
                        # there) to halve the PE den streams.
                        last_pair = jp == NP2 - 1

                        def den_mm(rhs_ap, stop):
                            nonlocal first_den
                            nc.tensor.matmul(
                                dacc[:], ones[:], rhs_ap,
                                start=first_den, stop=stop)
                            first_den = False

                        if masked or shadow:
                            if prev_dsum is not None:
                                den_mm(prev_dsum[:], False)
                                prev_dsum = None
                            den_mm(pt2[:, 0, :], False)
                            den_mm(pt2[:, 1, :], last_pair)
                        else:
                            dsum = att.tile([P, FD], BF16, tag="ds", name="ds")
                            nc.vector.tensor_add(
                                out=dsum[:], in0=pt2[:, 0, :], in1=pt2[:, 1, :])
                            if prev_dsum is None and not last_pair:
                                prev_dsum = dsum
                            else:
                                # fold two pair-sums into one den matmul
                                if prev_dsum is not None:
                                    qsum = att.tile([P, FD], BF16, tag="ds",
                                                    name="qs")
                                    nc.vector.tensor_add(
                                        out=qsum[:], in0=prev_dsum[:],
                                        in1=dsum[:])
                                    dsum = qsum
                                    prev_dsum = None
                                den_mm(dsum[:], last_pair)
                        if last_pair:
                            # free the single-bank accumulators ASAP: the next
                            # i-group's first PV/den matmuls wait on these
                            osb = att.tile([P, FD], F32, tag="osb",
                                           name="osb", bufs=2)
                            nc.vector.tensor_copy(osb[:], oacc[:])
                            dsb = evict_den(dacc)
                        if jp == fin_at and pending is not None:
                            finalize(pending)
                            pending = None
                    pending = (osb, dsb, h, gs)
            finalize(pending)

        # ---- Phase 3: output projection (partial over this core's heads) ----
        with (
            tc.tile_pool(name="pop", bufs=2, space="PSUM") as pop,
            tc.tile_pool(name="osp", bufs=3) as osp,
        ):
            for t in range(JC):
                pso = pop.tile([P, OUT_DIM], F32, tag="pop")
                for h in range(NH):
                    for nf in range(OUT_DIM // FD):
                        nc.tensor.matmul(
                            pso[:, nf * FD:(nf + 1) * FD],
                            ot[h][:, t * P:(t + 1) * P],
                            wo[:, h, nf * FD:(nf + 1) * FD],
                            start=(h == 0), stop=(h == NH - 1),
                        )
                ob = osp.tile([P, OUT_DIM], F32, tag="ob")
                # split the eviction across VectorE and ScalarE so neither
                # engine serializes the PSUM->SBUF drain behind the matmuls
                nc.vector.tensor_copy(ob[:, :FD], pso[:, :FD])
                nc.scalar.copy(ob[:, FD:], pso[:, FD:])
                nc.sync.dma_start(out_v[t], ob[:])

    nc.compile()
    return nc


def make_core_inputs(x, W_qkv, W_out, mask, n=N_FULL, mm=MM_FULL):
    """Host-side shard prep: per-core input dicts (bf16, pre-transposed).

    W slices are delivered in the on-chip layout ([128, c*h*d] with the
    IN_DIM chunk index between partition and column) so the DMA is dense.
    """
    bf = ml_dtypes.bfloat16
    B = x.shape[0]
    CI = IN_DIM // P
    xt_b = [np.ascontiguousarray(x[b].T).astype(bf) for b in range(B)]
    maskt = np.ascontiguousarray(mask[0, 0, :mm, :mm].T).astype(bf)

    def wlayout(w):  # [IN_DIM, NH*DH] -> [P, CI*NH*DH]
        return np.ascontiguousarray(
            w.reshape(CI, P, NH * DH).transpose(1, 0, 2).reshape(P, -1)
        ).astype(bf)

    cores_per_b = N_CORES // B
    in_maps = []
    for core in range(N_CORES):
        b = core // cores_per_b
        h0 = NH * (core % cores_per_b)
        qs, ks, vs = (W_qkv[:, o + h0 * DH: o + (h0 + NH) * DH]
                      for o in (0, OUT_DIM, 2 * OUT_DIM))
        wo_slice = W_out[h0 * DH:(h0 + NH) * DH, :]  # [NH*DH, OUT_DIM]
        wo_l = np.ascontiguousarray(
            wo_slice.reshape(NH, P, OUT_DIM).transpose(1, 0, 2).reshape(P, -1)
        ).astype(bf)
        in_maps.append({
            "xt": xt_b[b],
            "wq": wlayout(qs),
            "wk": wlayout(ks),
            "wv": wlayout(vs),
            "wo": wo_l,
            "maskt": maskt,
        })
    return in_maps


_NC_CACHE = {}


def _get_nc(n=N_FULL, mm=MM_FULL):
    key = (n, mm)
    if key not in _NC_CACHE:
        _NC_CACHE[key] = build_nc(n, mm)
    return _NC_CACHE[key]


def run(x, W_qkv, W_out, b_out, mask, trace=False, **trace_kwargs):
    nc = _get_nc()
    in_maps = make_core_inputs(x, W_qkv, W_out, mask)
    res = run_bass_kernel_spmd(
        nc, in_maps, list(range(N_CORES)), trace=trace, **trace_kwargs
    )
    B = x.shape[0]
    cores_per_b = N_CORES // B
    out = np.zeros((B, N_FULL, OUT_DIM), np.float32)
    for core in range(N_CORES):
        out[core // cores_per_b] += res.results[core]["part"]
    out += np.asarray(b_out, np.float32)
    return out, res


def kernel(x, W_qkv, W_out, b_out, mask, max_mask=MM_FULL, **_ignored):
    x = np.asarray(x, np.float32)
    W_qkv = np.asarray(W_qkv, np.float32)
    W_out = np.asarray(W_out, np.float32)
    b_out = np.asarray(b_out, np.float32)
    mask = np.asarray(mask)
    out, _ = run(x, W_qkv, W_out, b_out, mask)
    return out



# revision 2
# speedup vs baseline: 1.2071x; 1.2071x over previous
"""Bass/Trainium2 kernel for nn_Attention_369367188096 (sparse_attention).

Reference computation (B=2, N=4096, IN_DIM=1024, DIM=1024, HEADS=8, d=128):
    qkv = x @ W_qkv ; split into q,k,v per head
    dots = (q @ k^T) * DIM**-0.5 ; masked on top-left [2048,2048] block
    attn = softmax(dots) ; out = attn @ v ; out @ W_out + b_out

Sharding across 8 NeuronCores: core i handles batch b=i//4 and heads
(2*(i%4), 2*(i%4)+1).  Each core computes a partial output
x[b]-rows x DIM using its two heads' slice of W_out (row-sharded);
the host sums 4 partials per batch and adds b_out.

v2: ScalarE (ACT) exp is the fundamental floor (~N^2 elements/core at 1
elem/cycle/partition); everything else is arranged to keep ACT 100% fed
and to shrink PE work below the ACT pace:
- Projections stay bf16 (accuracy headroom); attention probabilities and
  V are fp8e4: the exp activation writes fp8 directly, and PV + softmax
  denominator run as fp8 DoubleRow matmuls over j-chunk PAIRS (2 k-planes
  per instruction), quartering those PE streams vs bf16.
- Denominator: ones-weights DoubleRow matmul leaves the row sum broadcast
  across partitions; 1/den via DVE reciprocal_approx_fast (5x faster than
  reciprocal, 18-bit accurate - plenty at 2e-2 tolerance).
- Mask is a 0/1 fp8 multiply on VectorE after exp (exactly exp then zero).
"""

import os
import sys

for _p in ("/opt/trn_rl_repo", "/root/.axon_site/_ro/trn_rl_repo"):
    if os.path.isdir(_p) and _p not in sys.path:
        sys.path.insert(0, _p)

from contextlib import ExitStack

import ml_dtypes
import numpy as np

import concourse.bass as bass
import concourse.bacc as bacc
import concourse.mybir as mybir
import concourse.tile as tile
from concourse.bass_utils import run_bass_kernel_spmd

BF16 = mybir.dt.bfloat16
FP8 = mybir.dt.float8e4
F32 = mybir.dt.float32
DR = mybir.MatmulPerfMode.DoubleRow
P = 128          # partitions
IN_DIM = 1024    # model in dim
OUT_DIM = 1024   # model out dim
DH = 128         # head dim
NH = 2           # heads per core
FD = 512         # matmul moving free dim
N_FULL = 4096    # sequence length
MM_FULL = 2048   # masked block size
SCALE = 1024 ** -0.5
N_CORES = 8


def build_nc(n=N_FULL, mm=MM_FULL):
    """Build the per-core Bass program (SPMD: same program, per-core data)."""
    CI = IN_DIM // P          # 8 input-dim chunks
    JC = n // P               # key chunks (32)
    IG = n // FD              # query groups of 512 (8)
    MJ = mm // P              # masked key chunks (16)
    MG = mm // FD             # masked query groups (4)
    assert MJ % 2 == 0 and JC % 2 == 0
    AF = mybir.ActivationFunctionType

    nc = bacc.Bacc("TRN2", target_bir_lowering=False, debug=False)
    # W tensors arrive host-prelayouted with 128 partitions contiguous so the
    # DMAs are dense and fast (they gate the first matmul).
    wq_d = nc.dram_tensor("wq", [P, CI * NH * DH], BF16, kind="ExternalInput")
    wk_d = nc.dram_tensor("wk", [P, CI * NH * DH], BF16, kind="ExternalInput")
    wv_d = nc.dram_tensor("wv", [P, CI * NH * DH], BF16, kind="ExternalInput")
    wo_d = nc.dram_tensor("wo", [P, NH * OUT_DIM], BF16, kind="ExternalInput")
    xt_d = nc.dram_tensor("xt", [IN_DIM, n], BF16, kind="ExternalInput")
    mk_d = nc.dram_tensor("maskt", [mm, mm], FP8, kind="ExternalInput")
    out_d = nc.dram_tensor("part", [n, OUT_DIM], F32, kind="ExternalOutput")

    xt_v = xt_d.rearrange("(c p) n -> c p n", p=P)
    mk_v = mk_d.rearrange("(j p) i -> p j i", p=P)
    out_v = out_d.rearrange("(t p) o -> t p o", p=P)

    with tile.TileContext(nc) as tc, ExitStack() as ctx:
        const = ctx.enter_context(tc.tile_pool(name="const", bufs=1))

        # Resident inputs (W first: they gate the first matmuls)
        wq = const.tile([P, CI, NH * DH], BF16, tag="wq")
        wk = const.tile([P, CI, NH * DH], BF16, tag="wk")
        wv = const.tile([P, CI, NH * DH], BF16, tag="wv")
        wo = const.tile([P, NH, OUT_DIM], BF16, tag="wo")
        for t, d_ in ((wq, wq_d), (wk, wk_d), (wv, wv_d), (wo, wo_d)):
            nc.sync.dma_start(t[:], d_.rearrange("p (a b) -> p a b", a=t.shape[1]))
        xt = [const.tile([P, n], BF16, tag=f"xt{c}", name=f"xt{c}") for c in range(CI)]
        for c in range(CI):
            nc.sync.dma_start(xt[c][:], xt_v[c])
        ones8 = const.tile([P, 2, P], FP8, tag="ones")
        nc.vector.memset(ones8[:], 1.0)

        # Resident intermediates
        qt = [const.tile([P, n], BF16, tag=f"qt{h}", name=f"qt{h}") for h in range(NH)]
        kt = [const.tile([P, n], BF16, tag=f"kt{h}", name=f"kt{h}") for h in range(NH)]
        vb8 = const.tile([P, JC, NH * DH], FP8, tag="vb")      # [j, jc, (h d)]
        ot = [const.tile([P, n], BF16, tag=f"ot{h}", name=f"ot{h}") for h in range(NH)]

        # ---- Phase 1: projections ----
        # Q^T, K^T per head: accumulate W[c,h].T @ x^T[c] over c.
        with tc.tile_pool(name="pq", bufs=4, space="PSUM") as pq:
            for h in range(NH):
                for w_sb, dst in ((wq, qt[h]), (wk, kt[h])):
                    for g0 in range(0, IG, 4):
                        gg = range(g0, min(g0 + 4, IG))
                        ps = [pq.tile([P, FD], F32, tag="pq", name="psqk") for _ in gg]
                        for c in range(CI):
                            for gi, g in enumerate(gg):
                                nc.tensor.matmul(
                                    ps[gi][:],
                                    w_sb[:, c, h * DH:(h + 1) * DH],
                                    xt[c][:, g * FD:(g + 1) * FD],
                                    start=(c == 0), stop=(c == CI - 1),
                                )
                        for gi, g in enumerate(gg):
                            nc.any.tensor_copy(dst[:, g * FD:(g + 1) * FD], ps[gi][:])
            # V (both heads) in natural [seq, d] layout: x^T[c] as weights.
            # Evicted straight to fp8 (V is only consumed by the fp8 PV).
            for t in range(JC):
                ps = pq.tile([P, NH * DH], F32, tag="pv")
                for c in range(CI):
                    nc.tensor.matmul(
                        ps[:], xt[c][:, t * P:(t + 1) * P], wv[:, c, :],
                        start=(c == 0), stop=(c == CI - 1),
                    )
                nc.any.tensor_copy(vb8[:, t, :], ps[:])

        # ---- Phase 2: attention per head ----
        # j-chunks processed in pairs: the pair is one wide exp on ScalarE
        # (fp8 out) and one fp8 DoubleRow matmul each for PV and den.
        with (
            tc.tile_pool(name="pst", bufs=3, space="PSUM") as pst,
            tc.tile_pool(name="po", bufs=1, space="PSUM") as po,
            tc.tile_pool(name="pd", bufs=1, space="PSUM") as pd,
            tc.tile_pool(name="att", bufs=8) as att,
            tc.tile_pool(name="mkp", bufs=8) as mkp,
        ):
            NP2 = JC // 2
            for h in range(NH):
                for g in range(IG):
                    gs = g * FD
                    oacc = po.tile([P, FD], F32, tag="po")   # [d, i] accum
                    dacc = pd.tile([P, FD], F32, tag="pd")   # bcast denom accum
                    for jp in range(NP2):
                        j0 = 2 * jp
                        st2 = pst.tile([P, 2, FD], F32, tag="st")
                        for u in range(2):
                            nc.tensor.matmul(
                                st2[:, u, :],
                                kt[h][:, (j0 + u) * P:(j0 + u + 1) * P],
                                qt[h][:, gs:gs + FD],
                                start=True, stop=True,
                            )
                        masked = j0 + 1 < MJ and g < MG
                        pt2 = att.tile([P, 2, FD], FP8, tag="pt")
                        nc.scalar.activation(pt2[:], st2[:], AF.Exp, scale=SCALE)
                        if masked:
                            mt2 = mkp.tile([P, 2, FD], FP8, tag="mt")
                            nc.sync.dma_start(
                                mt2[:], mk_v[:, j0:j0 + 2, gs:gs + FD])
                            nc.vector.tensor_mul(
                                out=pt2[:], in0=pt2[:], in1=mt2[:])
                        last_pair = jp == NP2 - 1
                        nc.tensor.matmul(
                            oacc[:], vb8[:, j0:j0 + 2, h * DH:(h + 1) * DH],
                            pt2[:], start=(jp == 0), stop=last_pair,
                            perf_mode=DR,
                        )
                        nc.tensor.matmul(
                            dacc[:], ones8[:], pt2[:],
                            start=(jp == 0), stop=last_pair,
                            perf_mode=DR,
                        )
                    # free the single-bank accumulators ASAP, then normalize
                    osb = att.tile([P, FD], F32, tag="osb", name="osb", bufs=2)
                    dsb = att.tile([P, FD], F32, tag="dsb", name="dsb", bufs=2)
                    nc.vector.tensor_copy(osb[:], oacc[:])
                    nc.vector.tensor_copy(dsb[:], dacc[:])
                    rec = att.tile([P, FD], F32, tag="rec", name="rec", bufs=2)
                    nc.vector.reciprocal_approx_fast(rec[:], dsb[:])
                    nc.vector.tensor_mul(
                        out=ot[h][:, gs:gs + FD], in0=osb[:], in1=rec[:])

        # ---- Phase 3: output projection (partial over this core's heads) ----
        with (
            tc.tile_pool(name="pop", bufs=2, space="PSUM") as pop,
            tc.tile_pool(name="osp", bufs=3) as osp,
        ):
            for t in range(JC):
                pso = pop.tile([P, OUT_DIM], F32, tag="pop")
                for h in range(NH):
                    for nf in range(OUT_DIM // FD):
                        nc.tensor.matmul(
                            pso[:, nf * FD:(nf + 1) * FD],
                            ot[h][:, t * P:(t + 1) * P],
                            wo[:, h, nf * FD:(nf + 1) * FD],
                            start=(h == 0), stop=(h == NH - 1),
                        )
                ob = osp.tile([P, OUT_DIM], F32, tag="ob")
                # split the eviction across VectorE and ScalarE so neither
                # engine serializes the PSUM->SBUF drain behind the matmuls
                nc.vector.tensor_copy(ob[:, :FD], pso[:, :FD])
                nc.scalar.copy(ob[:, FD:], pso[:, FD:])
                nc.sync.dma_start(out_v[t], ob[:])

    nc.compile()
    return nc


def make_core_inputs(x, W_qkv, W_out, mask, n=N_FULL, mm=MM_FULL):
    """Host-side shard prep: per-core input dicts (bf16, pre-transposed).

    W slices are delivered in the on-chip layout ([128, c*h*d] with the
    IN_DIM chunk index between partition and column) so the DMA is dense.
    """
    bf = ml_dtypes.bfloat16
    f8 = ml_dtypes.float8_e4m3
    B = x.shape[0]
    CI = IN_DIM // P
    xt_b = [np.ascontiguousarray(x[b].T).astype(bf) for b in range(B)]
    maskt = np.ascontiguousarray(mask[0, 0, :mm, :mm].T).astype(f8)

    def wlayout(w):  # [IN_DIM, NH*DH] -> [P, CI*NH*DH]
        return np.ascontiguousarray(
            w.reshape(CI, P, NH * DH).transpose(1, 0, 2).reshape(P, -1)
        ).astype(bf)

    cores_per_b = N_CORES // B
    in_maps = []
    for core in range(N_CORES):
        b = core // cores_per_b
        h0 = NH * (core % cores_per_b)
        qs, ks, vs = (W_qkv[:, o + h0 * DH: o + (h0 + NH) * DH]
                      for o in (0, OUT_DIM, 2 * OUT_DIM))
        wo_slice = W_out[h0 * DH:(h0 + NH) * DH, :]  # [NH*DH, OUT_DIM]
        wo_l = np.ascontiguousarray(
            wo_slice.reshape(NH, P, OUT_DIM).transpose(1, 0, 2).reshape(P, -1)
        ).astype(bf)
        in_maps.append({
            "xt": xt_b[b],
            "wq": wlayout(qs),
            "wk": wlayout(ks),
            "wv": wlayout(vs),
            "wo": wo_l,
            "maskt": maskt,
        })
    return in_maps


_NC_CACHE = {}


def _get_nc(n=N_FULL, mm=MM_FULL):
    key = (n, mm)
    if key not in _NC_CACHE:
        _NC_CACHE[key] = build_nc(n, mm)
    return _NC_CACHE[key]


def run(x, W_qkv, W_out, b_out, mask, trace=False, **trace_kwargs):
    nc = _get_nc()
    in_maps = make_core_inputs(x, W_qkv, W_out, mask)
    res = run_bass_kernel_spmd(
        nc, in_maps, list(range(N_CORES)), trace=trace, **trace_kwargs
    )
    B = x.shape[0]
    cores_per_b = N_CORES // B
    out = np.zeros((B, N_FULL, OUT_DIM), np.float32)
    for core in range(N_CORES):
        out[core // cores_per_b] += res.results[core]["part"]
    out += np.asarray(b_out, np.float32)
    return out, res


def kernel(x, W_qkv, W_out, b_out, mask, max_mask=MM_FULL, **_ignored):
    x = np.asarray(x, np.float32)
    W_qkv = np.asarray(W_qkv, np.float32)
    W_out = np.asarray(W_out, np.float32)
    b_out = np.asarray(b_out, np.float32)
    mask = np.asarray(mask)
    out, _ = run(x, W_qkv, W_out, b_out, mask)
    return out


# revision 3
# speedup vs baseline: 1.2998x; 1.0768x over previous
"""Bass/Trainium2 kernel for nn_Attention_369367188096 (sparse_attention).

Reference computation (B=2, N=4096, IN_DIM=1024, DIM=1024, HEADS=8, d=128):
    qkv = x @ W_qkv ; split into q,k,v per head
    dots = (q @ k^T) * DIM**-0.5 ; masked on top-left [2048,2048] block
    attn = softmax(dots) ; out = attn @ v ; out @ W_out + b_out

Sharding across 8 NeuronCores: core i handles batch b=i//4 and heads
(2*(i%4), 2*(i%4)+1).  Each core computes a partial output
x[b]-rows x DIM using its two heads' slice of W_out (row-sharded);
the host sums 4 partials per batch and adds b_out.

v3: PE-bound design, every non-S matmul stream shrunk and all engines
kept busy end-to-end:
- On real TRN2 a matmul costs out-free-size cycles regardless of dtype;
  fp8 DoubleRow's win is contracting TWO 128-deep k-planes per stream.
  PV and the softmax denominator contract j (4096) -> DR pairs halve
  them; Q/K projections contract IN_DIM (1024) -> DR over c-chunk pairs
  (x and W_q/W_k shipped as fp8; V projection stays bf16 for accuracy).
- S = K^T Q contracts only d=128, so it stays bf16 (no DR win exists).
- exp on ScalarE writes fp8 directly; mask is an fp8 0/1 multiply on
  VectorE; 1/den via DVE reciprocal_approx_fast.
- Single instruction stream interleaves the phases: V-projection chunks,
  head-1 Q/K projection units and output-projection halves are spliced
  into the attention pair loop's PE slack (in-order engine queues make
  emission order = execution order), so ScalarE's exp pipe starts ~25us
  in and the PE never idles long enough to drop out of its top p-state.
"""

import os
import sys

for _p in ("/opt/trn_rl_repo", "/root/.axon_site/_ro/trn_rl_repo"):
    if os.path.isdir(_p) and _p not in sys.path:
        sys.path.insert(0, _p)

from collections import deque
from contextlib import ExitStack

import ml_dtypes
import numpy as np

import concourse.bass as bass
import concourse.bacc as bacc
import concourse.mybir as mybir
import concourse.tile as tile
from concourse.bass_utils import run_bass_kernel_spmd

BF16 = mybir.dt.bfloat16
FP8 = mybir.dt.float8e4
F32 = mybir.dt.float32
DR = mybir.MatmulPerfMode.DoubleRow
P = 128          # partitions
IN_DIM = 1024    # model in dim
OUT_DIM = 1024   # model out dim
DH = 128         # head dim
NH = 2           # heads per core
FD = 512         # matmul moving free dim
N_FULL = 4096    # sequence length
MM_FULL = 2048   # masked block size
SCALE = 1024 ** -0.5
N_CORES = 8


def build_nc(n=N_FULL, mm=MM_FULL):
    """Build the per-core Bass program (SPMD: same program, per-core data)."""
    CI = IN_DIM // P          # 8 input-dim chunks
    CP = CI // 2              # c-chunk pairs for DR projections (4)
    JC = n // P               # key chunks (32)
    IG = n // FD              # query groups of 512 (8)
    MJ = mm // P              # masked key chunks (16)
    MG = mm // FD             # masked query groups (4)
    assert MJ % 2 == 0 and JC % 2 == 0
    AF = mybir.ActivationFunctionType

    nc = bacc.Bacc("TRN2", target_bir_lowering=False, debug=False)
    wq_d = nc.dram_tensor("wq", [P, CI * NH * DH], FP8, kind="ExternalInput")
    wk_d = nc.dram_tensor("wk", [P, CI * NH * DH], FP8, kind="ExternalInput")
    wv_d = nc.dram_tensor("wv", [P, CI * NH * DH], BF16, kind="ExternalInput")
    wo_d = nc.dram_tensor("wo", [P, NH * OUT_DIM], BF16, kind="ExternalInput")
    x8_d = nc.dram_tensor("x8", [IN_DIM, n], FP8, kind="ExternalInput")
    xt_d = nc.dram_tensor("xt", [IN_DIM, n], BF16, kind="ExternalInput")
    mk_d = nc.dram_tensor("maskt", [mm, mm], FP8, kind="ExternalInput")
    out_d = nc.dram_tensor("part", [n, OUT_DIM], F32, kind="ExternalOutput")

    x8_v = x8_d.rearrange("(c p) n -> c p n", p=P)
    xt_v = xt_d.rearrange("(c p) n -> c p n", p=P)
    mk_v = mk_d.rearrange("(j p) i -> p j i", p=P)
    out_v = out_d.rearrange("(t p) o -> t p o", p=P)

    with tile.TileContext(nc) as tc, ExitStack() as ctx:
        const = ctx.enter_context(tc.tile_pool(name="const", bufs=1))

        # Resident inputs, DMA'd in dependency order: fp8 QK path first
        # (it gates the first matmul), then the bf16 V path.
        wq8 = const.tile([P, CI, NH * DH], FP8, tag="wq")
        wk8 = const.tile([P, CI, NH * DH], FP8, tag="wk")
        for t, d_ in ((wq8, wq_d), (wk8, wk_d)):
            nc.sync.dma_start(t[:], d_.rearrange("p (a b) -> p a b", a=CI))
        x8 = const.tile([P, CI, n], FP8, tag="x8")
        for c in range(CI):
            nc.sync.dma_start(x8[:, c, :], x8_v[c])
        wv = const.tile([P, CI, NH * DH], BF16, tag="wv")
        wo = const.tile([P, NH, OUT_DIM], BF16, tag="wo")
        for t, d_ in ((wv, wv_d), (wo, wo_d)):
            nc.sync.dma_start(t[:], d_.rearrange("p (a b) -> p a b", a=t.shape[1]))
        xt = [const.tile([P, n], BF16, tag=f"xt{c}", name=f"xt{c}") for c in range(CI)]
        for c in range(CI):
            nc.sync.dma_start(xt[c][:], xt_v[c])
        ones8 = const.tile([P, 2, P], FP8, tag="ones")
        nc.vector.memset(ones8[:], 1.0)

        # Resident intermediates
        qt = [const.tile([P, n], BF16, tag=f"qt{h}", name=f"qt{h}") for h in range(NH)]
        kt = [const.tile([P, n], BF16, tag=f"kt{h}", name=f"kt{h}") for h in range(NH)]
        vb8 = const.tile([P, JC, NH * DH], FP8, tag="vb")      # [j, jc, (h d)]
        ot = [const.tile([P, n], BF16, tag=f"ot{h}", name=f"ot{h}") for h in range(NH)]

        pst = ctx.enter_context(tc.tile_pool(name="pst", bufs=2, space="PSUM"))
        px = ctx.enter_context(tc.tile_pool(name="px", bufs=2, space="PSUM"))
        po = ctx.enter_context(tc.tile_pool(name="po", bufs=1, space="PSUM"))
        pd = ctx.enter_context(tc.tile_pool(name="pd", bufs=1, space="PSUM"))
        att = ctx.enter_context(tc.tile_pool(name="att", bufs=5))
        mkp = ctx.enter_context(tc.tile_pool(name="mkp", bufs=4))
        obp = ctx.enter_context(tc.tile_pool(name="obp", bufs=3))

        # ---- emission units (each: a few PE streams + a DVE eviction) ----
        def emit_qk_g(h, w8, dst, g):
            # one i-group of a Q^T/K^T projection: DR over c-chunk pairs
            ps = px.tile([P, FD], F32, tag="u", name="psu")
            for cp in range(CP):
                nc.tensor.matmul(
                    ps[:], w8[:, 2 * cp:2 * cp + 2, h * DH:(h + 1) * DH],
                    x8[:, 2 * cp:2 * cp + 2, g * FD:(g + 1) * FD],
                    start=(cp == 0), stop=(cp == CP - 1), perf_mode=DR,
                )
            nc.vector.tensor_copy(dst[:, g * FD:(g + 1) * FD], ps[:])

        def emit_v_chunk(t):
            # one 128-row chunk of V for both heads (bf16), evicted to fp8
            ps = px.tile([P, FD], F32, tag="u", name="psu")
            pv = ps[:, :NH * DH]
            for c in range(CI):
                nc.tensor.matmul(
                    pv, xt[c][:, t * P:(t + 1) * P], wv[:, c, :],
                    start=(c == 0), stop=(c == CI - 1),
                )
            nc.vector.tensor_copy(vb8[:, t, :], pv)

        def emit_outproj_half(t, nf):
            ps = px.tile([P, FD], F32, tag="u", name="psu")
            for h in range(NH):
                nc.tensor.matmul(
                    ps[:], ot[h][:, t * P:(t + 1) * P],
                    wo[:, h, nf * FD:(nf + 1) * FD],
                    start=(h == 0), stop=(h == NH - 1),
                )
            ob = obp.tile([P, FD], F32, tag="ob", name="ob")
            nc.vector.tensor_copy(ob[:], ps[:])
            nc.sync.dma_start(out_v[t][:, nf * FD:(nf + 1) * FD], ob[:])

        # splice queue: (pe_cost_ns, emit_fn)
        splice = deque()

        # ---- head: Q/K projections for head 0, first V chunks ----
        for w8, dst in ((wq8, qt[0]), (wk8, kt[0])):
            for g in range(IG):
                emit_qk_g(0, w8, dst, g)
        V_UPFRONT = 4
        for t in range(V_UPFRONT):
            emit_v_chunk(t)
        v_todo = deque(range(V_UPFRONT, JC))
        for w8, dst in ((wq8, qt[1]), (wk8, kt[1])):
            for g in range(IG):
                splice.append((860, lambda h=1, w8=w8, dst=dst, g=g:
                               emit_qk_g(h, w8, dst, g)))

        # ---- attention pair loop (phases interleaved via splice pops) ----
        NP2 = JC // 2
        debt = [0.0]
        PAIR_SLACK = 190.0

        def pop_splices(force_v):
            if force_v:
                for _ in range(2):
                    if v_todo:
                        emit_v_chunk(v_todo.popleft())
            debt[0] += PAIR_SLACK
            while splice and debt[0] >= splice[0][0]:
                cost, fn = splice.popleft()
                debt[0] -= cost
                fn()

        for h in range(NH):
            for g in range(IG):
                gs = g * FD
                oacc = po.tile([P, FD], F32, tag="po")   # [d, i] accum
                dacc = pd.tile([P, FD], F32, tag="pd")   # bcast denom accum

                def emit_s(jp):
                    st = pst.tile([P, 2, FD], F32, tag="st", name="st")
                    for u in range(2):
                        nc.tensor.matmul(
                            st[:, u, :],
                            kt[h][:, (2 * jp + u) * P:(2 * jp + u + 1) * P],
                            qt[h][:, gs:gs + FD],
                            start=True, stop=True,
                        )
                    return st

                st_next = emit_s(0)
                for jp in range(NP2):
                    st = st_next
                    if jp + 1 < NP2:
                        st_next = emit_s(jp + 1)
                    pop_splices(force_v=(h == 0 and g == 0))
                    j0 = 2 * jp
                    masked = j0 + 1 < MJ and g < MG
                    pt2 = att.tile([P, 2, FD], FP8, tag="pt")
                    nc.scalar.activation(pt2[:], st[:], AF.Exp, scale=SCALE)
                    if masked:
                        mt2 = mkp.tile([P, 2, FD], FP8, tag="mt")
                        nc.sync.dma_start(
                            mt2[:], mk_v[:, j0:j0 + 2, gs:gs + FD])
                        nc.vector.tensor_mul(
                            out=pt2[:], in0=pt2[:], in1=mt2[:])
                    last_pair = jp == NP2 - 1
                    nc.tensor.matmul(
                        oacc[:], vb8[:, j0:j0 + 2, h * DH:(h + 1) * DH],
                        pt2[:], start=(jp == 0), stop=last_pair,
                        perf_mode=DR,
                    )
                    nc.tensor.matmul(
                        dacc[:], ones8[:], pt2[:],
                        start=(jp == 0), stop=last_pair,
                        perf_mode=DR,
                    )
                # free the single-bank accumulators ASAP, then normalize
                osb = att.tile([P, FD], F32, tag="osb", name="osb", bufs=2)
                dsb = att.tile([P, FD], F32, tag="dsb", name="dsb", bufs=2)
                nc.vector.tensor_copy(osb[:], oacc[:])
                nc.vector.tensor_copy(dsb[:], dacc[:])
                rec = att.tile([P, FD], F32, tag="rec", name="rec", bufs=2)
                nc.vector.reciprocal_approx_fast(rec[:], dsb[:])
                nc.vector.tensor_mul(
                    out=ot[h][:, gs:gs + FD], in0=osb[:], in1=rec[:])
                if h == NH - 1:
                    for t in range(4 * g, 4 * g + 4):
                        for nf in range(OUT_DIM // FD):
                            splice.append((440, lambda t=t, nf=nf:
                                           emit_outproj_half(t, nf)))

        # ---- tail: drain remaining spliced work ----
        while v_todo:
            emit_v_chunk(v_todo.popleft())
        while splice:
            splice.popleft()[1]()

    nc.compile()
    return nc


def make_core_inputs(x, W_qkv, W_out, mask, n=N_FULL, mm=MM_FULL):
    """Host-side shard prep: per-core input dicts (pre-transposed).

    W slices are delivered in the on-chip layout ([128, c*h*d] with the
    IN_DIM chunk index between partition and column) so the DMA is dense.
    x ships twice: fp8 (Q/K DoubleRow path) and bf16 (V path).
    """
    bf = ml_dtypes.bfloat16
    f8 = ml_dtypes.float8_e4m3
    B = x.shape[0]
    CI = IN_DIM // P
    xt_b = [np.ascontiguousarray(x[b].T) for b in range(B)]
    xt_bf = [t.astype(bf) for t in xt_b]
    xt_f8 = [t.astype(f8) for t in xt_b]
    maskt = np.ascontiguousarray(mask[0, 0, :mm, :mm].T).astype(f8)

    def wlayout(w, dt):  # [IN_DIM, NH*DH] -> [P, CI*NH*DH]
        return np.ascontiguousarray(
            w.reshape(CI, P, NH * DH).transpose(1, 0, 2).reshape(P, -1)
        ).astype(dt)

    cores_per_b = N_CORES // B
    in_maps = []
    for core in range(N_CORES):
        b = core // cores_per_b
        h0 = NH * (core % cores_per_b)
        qs, ks, vs = (W_qkv[:, o + h0 * DH: o + (h0 + NH) * DH]
                      for o in (0, OUT_DIM, 2 * OUT_DIM))
        wo_slice = W_out[h0 * DH:(h0 + NH) * DH, :]  # [NH*DH, OUT_DIM]
        wo_l = np.ascontiguousarray(
            wo_slice.reshape(NH, P, OUT_DIM).transpose(1, 0, 2).reshape(P, -1)
        ).astype(bf)
        in_maps.append({
            "xt": xt_bf[b],
            "x8": xt_f8[b],
            "wq": wlayout(qs, f8),
            "wk": wlayout(ks, f8),
            "wv": wlayout(vs, bf),
            "wo": wo_l,
            "maskt": maskt,
        })
    return in_maps


_NC_CACHE = {}


def _get_nc(n=N_FULL, mm=MM_FULL):
    key = (n, mm)
    if key not in _NC_CACHE:
        _NC_CACHE[key] = build_nc(n, mm)
    return _NC_CACHE[key]


def run(x, W_qkv, W_out, b_out, mask, trace=False, **trace_kwargs):
    nc = _get_nc()
    in_maps = make_core_inputs(x, W_qkv, W_out, mask)
    res = run_bass_kernel_spmd(
        nc, in_maps, list(range(N_CORES)), trace=trace, **trace_kwargs
    )
    B = x.shape[0]
    cores_per_b = N_CORES // B
    out = np.zeros((B, N_FULL, OUT_DIM), np.float32)
    for core in range(N_CORES):
        out[core // cores_per_b] += res.results[core]["part"]
    out += np.asarray(b_out, np.float32)
    return out, res


def kernel(x, W_qkv, W_out, b_out, mask, max_mask=MM_FULL, **_ignored):
    x = np.asarray(x, np.float32)
    W_qkv = np.asarray(W_qkv, np.float32)
    W_out = np.asarray(W_out, np.float32)
    b_out = np.asarray(b_out, np.float32)
    mask = np.asarray(mask)
    out, _ = run(x, W_qkv, W_out, b_out, mask)
    return out
